# revision 10
# baseline (speedup 1.0000x reference)
"""Trainium2 Bass kernel for nn_ExLoss (exemplar-memory loss).

Math (see reference):
    outputs = (inputs @ V.T) * T                # [B, C], T = 1.0
    sims    = (inputs / ||inputs||) @ V.T       # row-scaled `outputs`
    tsims   = V[targets] @ V.T                  # [B, C]
    loss    = masked-mean softplus over tsims gathered at neg-pair columns

All heavy FLOPs live in two GEMMs sharing V.T as one operand, and sims is just
a row scaling of outputs.  So the device computes one fused GEMM
    OUT = [inputs; V[targets]] @ V.T            # [512, C]
sharded column-wise (class dim) across 8 NeuronCores: each core holds
V_s.T = [D, C/8] and computes OUT_s = X @ V_s.T = [512, 2048].

Layout: the tensor engine contracts over the partition dim, so both operands
are fed D-major (X.T and V_s.T built host-side).  Operands use float32r
(~TF32 precision on the PE, 4x faster than fp32) with fp32 PSUM accumulation;
measured max rel err ~2e-4 for K=2048 dots.

Device loop per core: k split into 4 groups of 4x128; within a group each of
the 16 [128m, 512n] output tiles accumulates 4 matmuls in PSUM, then one DVE
op folds the group into an SBUF f32 accumulator (copy / add / add+store).
This keeps at most 8 PSUM banks live, streams V exactly once from HBM, and
keeps the PE busy behind the DMA stream.

The remaining loss math touches only [B, NPAIRS] gathered values and is done
on host in fp32, mirroring the reference expression exactly.
"""

import numpy as np
from contextlib import ExitStack

import concourse.bacc as bacc
import concourse.mybir as mybir
import concourse.tile as tile
from concourse.bass_utils import run_bass_kernel_spmd

B, D, C, NPAIRS = 256, 2048, 16384, 64
T = 1.0
N_MARGIN = 0.3
EPS = 1e-12

NCORES = 8
CS = C // NCORES          # 2048 class columns per core
XR = 2 * B                # 512 rows of the fused GEMM
P = 128
KT = D // P               # 16 k-tiles
# uneven k-groups: small final group so output DMA (4.2 MB burst) overlaps
# the last group's PE span instead of serializing after it
KGROUPS = [3, 5, 5, 3]
MT = XR // P              # 4 m-tiles
NT = CS // 512            # 4 n-tiles of 512

_compiled = None


def _build():
    global _compiled
    if _compiled is not None:
        return _compiled

    nc = bacc.Bacc("TRN2", target_bir_lowering=False, debug=False)
    f32 = mybir.dt.float32
    f32r = mybir.dt.float32r

    xt = nc.dram_tensor("xt", [D, XR], f32r, kind="ExternalInput").ap()
    vt = nc.dram_tensor("vt", [D, CS], f32r, kind="ExternalInput").ap()
    out = nc.dram_tensor("out", [XR, CS], f32, kind="ExternalOutput").ap()

    with tile.TileContext(nc) as tc, ExitStack() as ctx:
        vpool = ctx.enter_context(tc.tile_pool(name="vpool", bufs=13))
        xpool = ctx.enter_context(tc.tile_pool(name="xpool", bufs=16))
        apool = ctx.enter_context(tc.tile_pool(name="apool", bufs=1))
        opool = ctx.enter_context(tc.tile_pool(name="opool", bufs=6))
        ppool = ctx.enter_context(tc.tile_pool(name="ppool", bufs=8, space="PSUM"))

        # persistent f32 accumulators, one per m-tile, covering all CS columns
        acc = [apool.tile([P, CS], f32, name=f"acc{m}") for m in range(MT)]

        k0 = 0
        for kg, KJ in enumerate(KGROUPS):
            vts, xts = [], []
            for j in range(KJ):
                k = k0 + j
                xt_t = xpool.tile([P, XR], f32r, name=f"xt_{k}", tag="xt")
                nc.sync.dma_start(xt_t[:], xt[k * P:(k + 1) * P, :])
                vt_t = vpool.tile([P, CS], f32r, name=f"vt_{k}", tag="vt")
                nc.sync.dma_start(vt_t[:], vt[k * P:(k + 1) * P, :])
                vts.append(vt_t)
                xts.append(xt_t)
            k0 += KJ

            for m in range(MT):
                for n in range(NT):
                    ps = ppool.tile([P, 512], f32, name=f"ps_{kg}_{m}_{n}", tag="ps")
                    for j in range(KJ):
                        nc.tensor.matmul(
                            ps[:],
                            xts[j][:, m * P:(m + 1) * P],
                            vts[j][:, n * 512:(n + 1) * 512],
                            start=(j == 0),
                            stop=(j == KJ - 1),
                        )
                    a = acc[m][:, n * 512:(n + 1) * 512]
                    if kg == 0:
                        nc.vector.tensor_copy(a, ps[:])
                    elif kg < len(KGROUPS) - 1:
                        nc.vector.tensor_add(a, a, ps[:])
                    else:
                        o = opool.tile([P, 512], f32, name=f"o_{m}_{n}", tag="o")
                        nc.vector.tensor_add(o[:], a, ps[:])
                        nc.sync.dma_start(
                            out[m * P:(m + 1) * P, n * 512:(n + 1) * 512], o[:]
                        )

    nc.compile()
    _compiled = nc
    return nc


def _run_device(in_maps, trace=False):
    nc = _build()
    last = None
    for attempt in range(3):
        try:
            return run_bass_kernel_spmd(
                nc, in_maps, core_ids=list(range(NCORES)), trace=trace,
                trace_cores=list(range(NCORES)) if trace else None,
            )
        except Exception as e:
            # the axon-tunneled runtime occasionally reports the exec unit
            # unrecoverable on the launch after a profiled session; the
            # failing attempt resets the device, so retry after a pause
            last = e
            import time as _time
            _time.sleep(5 * (attempt + 1))
    raise last


def _make_in_maps(inputs, V, targets):
    X = np.empty((XR, D), dtype=np.float32)
    X[:B] = inputs
    X[B:] = V[targets]
    XT = np.ascontiguousarray(X.T)
    in_maps = []
    for s in range(NCORES):
        VTs = np.ascontiguousarray(V[s * CS:(s + 1) * CS, :].T)
        in_maps.append({"xt": XT, "vt": VTs})
    return in_maps


def kernel(inputs, targets, neg_pairs, indexs, all_label_to_clusterid, V,
           _trace=False, _results_out=None):
    inputs = np.asarray(inputs, dtype=np.float32)
    V = np.asarray(V, dtype=np.float32)
    tgt = np.asarray(targets).astype(np.int64)

    in_maps = _make_in_maps(inputs, V, tgt)
    res = _run_device(in_maps, trace=_trace)
    if _results_out is not None:
        _results_out.append(res)

    full = np.empty((XR, C), dtype=np.float32)
    for s in range(NCORES):
        full[:, s * CS:(s + 1) * CS] = res.results[s]["out"]

    outputs = full[:B] * np.float32(T)
    tsims = full[B:]

    # loss (host, fp32, mirrors reference exactly on [B, NPAIRS] data)
    norms = np.maximum(np.sqrt(np.sum(inputs * inputs, axis=1)), np.float32(EPS))
    n_thrds = outputs[np.arange(B), tgt] / norms - np.float32(N_MARGIN)  # [B]

    cids = np.asarray(all_label_to_clusterid).astype(np.int64)[
        np.asarray(neg_pairs).astype(np.int64)
    ]                                                                    # [B, N]
    nsims = np.take_along_axis(tsims, cids, axis=1)                      # [B, N]
    mask = (nsims > n_thrds[:, None]) & (nsims < np.float32(0.999999))
    count = np.float32(mask.sum())
    bce0 = np.logaddexp(np.float32(0.0), nsims).astype(np.float32)       # softplus
    hn_loss = (
        np.float32(np.where(mask, bce0, np.float32(0.0)).sum())
        / np.maximum(count, np.float32(1.0))
        if count > 0
        else np.float32(0.0)
    )
    th_loss = hn_loss if np.all(V.sum(axis=1) != 0) else np.float32(0.0)
    loss = np.float32(1.0) * th_loss

    return np.float32(loss), outputs


# revision 12
# speedup vs baseline: 1.0574x; 1.0574x over previous
"""Trainium2 Bass kernel for nn_ExLoss (exemplar-memory loss).

Math (see reference):
    outputs = (inputs @ V.T) * T                # [B, C], T = 1.0
    sims    = (inputs / ||inputs||) @ V.T       # row-scaled `outputs`
    tsims   = V[targets] @ V.T                  # [B, C]
    loss    = masked-mean softplus over tsims gathered at neg-pair columns

All heavy FLOPs live in two GEMMs sharing V.T as one operand, and sims is just
a row scaling of outputs.  So the device computes one fused GEMM
    OUT = [inputs; V[targets]] @ V.T            # [512, C]
sharded column-wise (class dim) across 8 NeuronCores: each core holds
V_s.T = [D, C/8] and computes OUT_s = X @ V_s.T = [512, 2048].

Layout: the tensor engine contracts over the partition dim, so both operands
are fed D-major (X.T and V_s.T built host-side).  Operands use float32r
(~TF32 precision on the PE, 4x faster than fp32) with fp32 PSUM accumulation;
measured max rel err ~2e-4 for K=2048 dots.

Device loop per core: k split into 4 groups of 4x128; within a group each of
the 16 [128m, 512n] output tiles accumulates 4 matmuls in PSUM, then one DVE
op folds the group into an SBUF f32 accumulator (copy / add / add+store).
This keeps at most 8 PSUM banks live, streams V exactly once from HBM, and
keeps the PE busy behind the DMA stream.

The remaining loss math touches only [B, NPAIRS] gathered values and is done
on host in fp32, mirroring the reference expression exactly.
"""

import numpy as np
from contextlib import ExitStack

import concourse.bacc as bacc
import concourse.mybir as mybir
import concourse.tile as tile
from concourse.bass_utils import run_bass_kernel_spmd

B, D, C, NPAIRS = 256, 2048, 16384, 64
T = 1.0
N_MARGIN = 0.3
EPS = 1e-12

NCORES = 8
CS = C // NCORES          # 2048 class columns per core
XR = 2 * B                # 512 rows of the fused GEMM
P = 128
KT = D // P               # 16 k-tiles
# uneven k-groups: size-1 first group lets the PE start on the first vt tile
# (no head-of-line wait), small final group so the output DMA burst overlaps
# the last group's PE span instead of serializing after it
KGROUPS = [1, 3, 4, 5, 3]
MT = XR // P              # 4 m-tiles
NT = CS // 512            # 4 n-tiles of 512

_compiled = None



def _build():
    global _compiled
    if _compiled is not None:
        return _compiled

    nc = bacc.Bacc("TRN2", target_bir_lowering=False, debug=False)
    f32 = mybir.dt.float32
    f32r = mybir.dt.float32r

    xt = nc.dram_tensor("xt", [D, XR], f32r, kind="ExternalInput").ap()
    vt = nc.dram_tensor("vt", [D, CS], f32r, kind="ExternalInput").ap()
    out = nc.dram_tensor("out", [XR, CS], f32, kind="ExternalOutput").ap()

    with tile.TileContext(nc) as tc, ExitStack() as ctx:
        vpool = ctx.enter_context(tc.tile_pool(name="vpool", bufs=12))
        xpool = ctx.enter_context(tc.tile_pool(name="xpool", bufs=16))
        apool = ctx.enter_context(tc.tile_pool(name="apool", bufs=1))
        opool = ctx.enter_context(tc.tile_pool(name="opool", bufs=6))
        ppool = ctx.enter_context(tc.tile_pool(name="ppool", bufs=8, space="PSUM"))

        # persistent f32 accumulators, one per m-tile, covering all CS columns
        acc = [apool.tile([P, CS], f32, name=f"acc{m}") for m in range(MT)]

        k0 = 0
        for kg, KJ in enumerate(KGROUPS):
            vts, xts = [], []
            for j in range(KJ):
                k = k0 + j
                xt_t = xpool.tile([P, XR], f32r, name=f"xt_{k}", tag="xt")
                nc.sync.dma_start(xt_t[:], xt[k * P:(k + 1) * P, :])
                vt_t = vpool.tile([P, CS], f32r, name=f"vt_{k}", tag="vt")
                nc.sync.dma_start(vt_t[:], vt[k * P:(k + 1) * P, :])
                vts.append(vt_t)
                xts.append(xt_t)
            k0 += KJ

            for m in range(MT):
                for n in range(NT):
                    ps = ppool.tile([P, 512], f32, name=f"ps_{kg}_{m}_{n}", tag="ps")
                    for j in range(KJ):
                        nc.tensor.matmul(
                            ps[:],
                            xts[j][:, m * P:(m + 1) * P],
                            vts[j][:, n * 512:(n + 1) * 512],
                            start=(j == 0),
                            stop=(j == KJ - 1),
                        )
                    a = acc[m][:, n * 512:(n + 1) * 512]
                    if kg == 0:
                        nc.vector.tensor_copy(a, ps[:])
                    elif kg < len(KGROUPS) - 1:
                        nc.vector.tensor_add(a, a, ps[:])
                    else:
                        o = opool.tile([P, 512], f32, name=f"o_{m}_{n}", tag="o")
                        nc.vector.tensor_add(o[:], a, ps[:])
                        nc.sync.dma_start(
                            out[m * P:(m + 1) * P, n * 512:(n + 1) * 512], o[:]
                        )

    nc.compile()
    _compiled = nc
    return nc


def _run_device(in_maps, trace=False):
    nc = _build()
    last = None
    for attempt in range(3):
        try:
            return run_bass_kernel_spmd(
                nc, in_maps, core_ids=list(range(NCORES)), trace=trace,
                trace_cores=list(range(NCORES)) if trace else None,
            )
        except Exception as e:
            # the axon-tunneled runtime occasionally reports the exec unit
            # unrecoverable on the launch after a profiled session; the
            # failing attempt resets the device, so retry after a pause
            last = e
            import time as _time
            _time.sleep(5 * (attempt + 1))
    raise last


def _make_in_maps(inputs, V, targets):
    X = np.empty((XR, D), dtype=np.float32)
    X[:B] = inputs
    X[B:] = V[targets]
    XT = np.ascontiguousarray(X.T)
    in_maps = []
    for s in range(NCORES):
        VTs = np.ascontiguousarray(V[s * CS:(s + 1) * CS, :].T)
        in_maps.append({"xt": XT, "vt": VTs})
    return in_maps


def kernel(inputs, targets, neg_pairs, indexs, all_label_to_clusterid, V,
           _trace=False, _results_out=None):
    inputs = np.asarray(inputs, dtype=np.float32)
    V = np.asarray(V, dtype=np.float32)
    tgt = np.asarray(targets).astype(np.int64)

    in_maps = _make_in_maps(inputs, V, tgt)
    res = _run_device(in_maps, trace=_trace)
    if _results_out is not None:
        _results_out.append(res)

    full = np.empty((XR, C), dtype=np.float32)
    for s in range(NCORES):
        full[:, s * CS:(s + 1) * CS] = res.results[s]["out"]

    outputs = full[:B] * np.float32(T)
    tsims = full[B:]

    # loss (host, fp32, mirrors reference exactly on [B, NPAIRS] data)
    norms = np.maximum(np.sqrt(np.sum(inputs * inputs, axis=1)), np.float32(EPS))
    n_thrds = outputs[np.arange(B), tgt] / norms - np.float32(N_MARGIN)  # [B]

    cids = np.asarray(all_label_to_clusterid).astype(np.int64)[
        np.asarray(neg_pairs).astype(np.int64)
    ]                                                                    # [B, N]
    nsims = np.take_along_axis(tsims, cids, axis=1)                      # [B, N]
    mask = (nsims > n_thrds[:, None]) & (nsims < np.float32(0.999999))
    count = np.float32(mask.sum())
    bce0 = np.logaddexp(np.float32(0.0), nsims).astype(np.float32)       # softplus
    hn_loss = (
        np.float32(np.where(mask, bce0, np.float32(0.0)).sum())
        / np.maximum(count, np.float32(1.0))
        if count > 0
        else np.float32(0.0)
    )
    th_loss = hn_loss if np.all(V.sum(axis=1) != 0) else np.float32(0.0)
    loss = np.float32(1.0) * th_loss

    return np.float32(loss), outputs
